# revision 9
# baseline (speedup 1.0000x reference)
"""Trainium2 Bass kernel for nn_CrossAttention_44693429682227.

Math (reference):
    q = (x @ Wq.T) / E**0.25, reshaped (b, t, H, E)
    scores = q @ keys.T over a shared bank of N=50000 (key, scalar-value) pairs
    attn = softmax(scores, axis=-1)
    out = mean_h(attn @ values) + curiosity  -> (b, t, 1)

Because values are scalars, out_row = (sum_n exp(s_n) * v_n) / (sum_n exp(s_n)).
Scores are bounded (|s| <~ 20), so exp never overflows; numerator and
denominator partials are exact to merge across key-bank shards.

Distribution: the key bank is sharded 8 ways (6528 keys/core); every core
computes the projection for all 4096 (b,t,h) query rows (replicated, cheap)
and partial num/den sums over its shard. Host merges partials.

Per-core program (all static/unrolled, Tile-scheduled), per head:
  - PE: qT projection (pipelined one head ahead), then per key-block trio:
        scoresT = keysT_block.T @ qT_head   (f32r, [128 keys, 512 qrows])
  - exp is split across TWO engines (both read scores from PSUM):
      * ACT: exp -> bf16, groups of 3 PSUM banks
      * DVE: Schraudolph bitcast exp -> int16 == bf16 bits, one
        tensor_scalar (round(s*128*log2e + (127-c)*128)); ~1.8% rms log
        error on the assigned fraction of key-blocks (softmax tolerates it)
  - num/den: column-TILED bf16 matmuls (tile_position=(0,32*(b%4))):
        ndacc[32s:32s+32] += vb_block.T @ eT_block, 4 strips concurrent on
        the PE array, so the reduction costs ~1/3 of its streaming time.
        vb = [v_hi, v_lo, mask, 0...] (32 stationary cols; walrus requires
        full-tile stationary width and non-f32r dtype for tiling).
  - nd trios lag scores by LAG groups (across head boundaries) so PE never
    stalls on ACT/DVE latency; per-head [128,512] nd bank -> SBUF -> DRAM.

kernel.py is self-contained: shapes/sharding hardcoded, no sibling imports.
"""

import os
import sys
from contextlib import ExitStack

import numpy as np

if "/opt/trn_rl_repo" not in sys.path:
    sys.path.insert(0, "/opt/trn_rl_repo")

import ml_dtypes

# Problem shapes (hardcoded per contract)
B, T = 4, 128
BT = B * T            # 512 query (b,t) rows
HIN = 1024
H, E = 8, 128
N = 50000
NCORES = 8

# Sharding / tiling
GB = 3                # key-blocks (128 keys each) per exp group (3 PSUM banks)
NGRP = 17             # groups per core
NBLK = GB * NGRP      # 51 key-blocks per core
KC = NBLK * 128       # 6528 keys per core
NPAD = KC * NCORES    # 52224 padded bank size
KCH = HIN // 128      # 8 contraction chunks for the projection
NSTRIP = 4            # nd col-tile strips
VBW = 32              # stationary width for col-tiled nd matmuls

# knobs
# Every exp group [128, GB*512] is split column-wise between the two exp
# engines: ACT takes cols [0:XACT] (true exp), DVE takes [XACT:] (Schraudolph
# bitcast exp). Both drain the same PSUM tile concurrently, so ps_s only
# needs 2 buffers. XACT balances ACT (0.833 ns/col + 293 ns/op) against
# DVE (1.042 ns/col + 157 ns/op).
XACT = int(os.environ.get("KXACT", "820"))
LAG = int(os.environ.get("KLAG", "8"))       # nd trio lag in groups
NDB = int(os.environ.get("KNDB", "8"))       # nd trio flush batch (groups); batches
                                             # amortize the PE tiling-mode switch
TRACE = bool(int(os.environ.get("KTRACE", "0")))

# Schraudolph constants (int16 -> bf16 bitcast, DVE convert = round-nearest)
SCH_A = float(128.0 / np.log(2.0))
SCH_C = 0.0575
SCH_B = float((127.0 - SCH_C) * 128.0)

LAST_RESULTS = None   # BassKernelResults of the most recent run (for test.py)

_cache = {}


def _install_ntff_hook():
    """Register the axon NTFF profile hook that this image's antenv lacks."""
    import types

    if "antenv.axon_hooks" in sys.modules:
        return
    try:
        from trn_agent_boot.trn_boot import _ntff_profile_via_ctypes

        hook = _ntff_profile_via_ctypes("/opt/axon/libaxon_pjrt.so")
    except Exception:
        hook = None
    mod = types.ModuleType("antenv.axon_hooks")
    mod.get_axon_ntff_profile_hook = lambda: hook
    sys.modules["antenv.axon_hooks"] = mod

    from concourse import bass_utils as bu

    orig_upload = bu.upload_artifacts

    def safe_upload(tmpdir):
        try:
            return orig_upload(tmpdir)
        except Exception as e:
            return f"upload-skipped ({type(e).__name__})"

    bu.upload_artifacts = safe_upload


def _dve_groups():
    """Evenly spread NDVE of NGRP groups for the DVE engine."""
    if NDVE <= 0:
        return set()
    pick = set()
    for k in range(NDVE):
        pick.add(int(round((k + 0.5) * NGRP / NDVE)) % NGRP)
    # rounding collisions: fill greedily
    k = 0
    while len(pick) < NDVE:
        if k not in pick:
            pick.add(k)
        k += 1
    return pick


def _build():
    import concourse.bass as bass
    import concourse.tile as tile
    from concourse import bacc, mybir

    f32 = mybir.dt.float32
    f32r = mybir.dt.float32r
    bf16 = mybir.dt.bfloat16
    i16 = mybir.dt.int16

    nc = bacc.Bacc(trn_type="TRN2", target_bir_lowering=False, debug=False)

    # Host pre-arranges xt/wqt so every DMA is one contiguous run per
    # partition: xt[p, k, bt] = x[bt, 128k+p]; wqt[h, p, k, e] = Wq.T[128k+p, 128h+e]
    xt_d = nc.dram_tensor("xt", [128, KCH * BT], f32r, kind="ExternalInput")
    wqt_d = nc.dram_tensor("wqt", [H, 128, KCH * E], f32r, kind="ExternalInput")
    keyst_d = nc.dram_tensor("keyst", [E, KC], bf16, kind="ExternalInput")
    vb_d = nc.dram_tensor("vb", [128, NBLK * VBW], bf16, kind="ExternalInput")
    nd_d = nc.dram_tensor("nd_out", [128, H * BT], f32, kind="ExternalOutput")

    Exp = mybir.ActivationFunctionType.Exp
    mult = mybir.AluOpType.mult
    add = mybir.AluOpType.add

    with tile.TileContext(nc) as tc, ExitStack() as ctx:
        singles = ctx.enter_context(tc.tile_pool(name="singles", bufs=1))
        epool = ctx.enter_context(tc.tile_pool(name="epool", bufs=NDB + LAG + 3))
        ps_s = ctx.enter_context(tc.tile_pool(name="ps_s", bufs=2, space="PSUM"))
        ps_q = ctx.enter_context(tc.tile_pool(name="ps_q", bufs=1, space="PSUM"))
        ps_nd = ctx.enter_context(tc.tile_pool(name="ps_nd", bufs=1, space="PSUM"))

        # ---- persistent SBUF loads, critical-path-first ----
        def load(pool, name, shape, dtype, src):
            t = pool.tile(shape, dtype, name=name, tag=name)
            nc.sync.dma_start(out=t, in_=src)
            return t

        wq_h = [None] * H
        xt_k = [None] * KCH
        keyst_c = [None] * NGRP
        KCH_G = GB * 128  # keys per group-chunk

        def load_wq(h):
            wq_h[h] = load(
                singles, f"wq{h}", [128, KCH, E], f32r,
                wqt_d.ap()[h].rearrange("p (k e) -> p k e", e=E),
            )

        def load_wq0():
            wq_h[0] = load(
                singles, "wq0", [128, KCH, E], f32r,
                wqt_d.ap()[0].rearrange("p (k e) -> p k e", e=E),
            )
            xt = load(
                singles, "xt", [128, KCH, BT], f32r,
                xt_d.ap().rearrange("p (k b) -> p k b", b=BT),
            )
            for k in range(KCH):
                xt_k[k] = xt[:, k, :]

        def load_kc(i):
            keyst_c[i] = load(
                singles, f"keyst{i}", [128, KCH_G], bf16,
                keyst_d.ap()[:, KCH_G * i:KCH_G * (i + 1)],
            )

        load_wq0()
        for i in range(4):
            load_kc(i)
        vb_sb = load(
            singles, "vb", [128, NBLK, VBW], bf16,
            vb_d.ap().rearrange("p (b c) -> p b c", c=VBW),
        )
        load_wq(1)
        for i in range(4, 9):
            load_kc(i)
        load_wq(2)
        for i in range(9, 13):
            load_kc(i)
        load_wq(3)
        for i in range(13, NGRP):
            load_kc(i)
        for h in range(4, H):
            load_wq(h)

        qt_sb = singles.tile([128, H, BT], bf16)
        out_sb = singles.tile([128, H, BT], f32)

        def proj(h):
            q_ps = ps_q.tile([128, BT], f32, tag="q", name=f"q_ps{h}")
            for k in range(KCH):
                nc.tensor.matmul(
                    q_ps,
                    lhsT=wq_h[h][:, k, :],
                    rhs=xt_k[k],
                    start=(k == 0),
                    stop=(k == KCH - 1),
                )
            nc.vector.tensor_copy(qt_sb[:, h, :], q_ps)

        # strip boundaries for nd accumulation flags
        first_b = {s: s for s in range(NSTRIP)}
        last_b = {s: max(b for b in range(NBLK) if b % NSTRIP == s)
                  for s in range(NSTRIP)}

        eT = {}        # (h, g) -> bf16-viewable eT tile
        nd_tile = [None] * H

        def nd_trio(h, g):
            if nd_tile[h] is None:
                nd_tile[h] = ps_nd.tile([128, BT], f32, tag="nd", name=f"nd{h}")
            ndp = nd_tile[h]
            et = eT.pop((h, g))
            for j in range(GB):
                b = g * GB + j
                s = b % NSTRIP
                nc.tensor.matmul(
                    ndp[32 * s:32 * s + VBW, :],
                    lhsT=vb_sb[:, b, :],
                    rhs=et[:, BT * j:BT * (j + 1)],
                    start=(b == first_b[s]),
                    stop=(b == last_b[s]),
                    tile_position=(0, 32 * s),
                )
            if g == NGRP - 1:
                nc.vector.tensor_copy(out_sb[:, h, :], ndp)
                nc.sync.dma_start(
                    out=nd_d.ap()[:, h * BT:(h + 1) * BT], in_=out_sb[:, h, :]
                )

        pending = []

        proj(0)
        for h in range(H):
            for g in range(NGRP):
                s_ps = ps_s.tile([128, GB * BT], f32, tag="s", name=f"s_ps_{h}_{g}")
                for j in range(GB):
                    nc.tensor.matmul(
                        s_ps[:, BT * j:BT * (j + 1)],
                        lhsT=keyst_c[g][:, 128 * j:128 * (j + 1)],
                        rhs=qt_sb[:, h, :],
                        start=True,
                        stop=True,
                    )
                et = epool.tile([128, GB * BT], bf16, tag="e", name=f"eT_{h}_{g}")
                nc.scalar.activation(et[:, :XACT], s_ps[:, :XACT], Exp)
                nc.vector.tensor_scalar(
                    et[:, XACT:].bitcast(i16), s_ps[:, XACT:],
                    SCH_A, SCH_B, mult, add,
                )
                eT[(h, g)] = et
                pending.append((h, g))
                if len(pending) >= NDB + LAG:
                    for _ in range(NDB):
                        nd_trio(*pending.pop(0))
                if g == 0 and h + 1 < H:
                    proj(h + 1)
        while pending:
            nd_trio(*pending.pop(0))

    nc.compile()
    return nc


def _prep_inputs(x, Wq, keys, values):
    bf = ml_dtypes.bfloat16

    # xt[p, k, bt] = x[bt, 128k+p]  (one contiguous run per partition)
    xT = np.ascontiguousarray(
        np.asarray(x, dtype=np.float32).reshape(BT, KCH, 128).transpose(2, 1, 0)
    ).reshape(128, KCH * BT)
    # wqt[h, p, k, e] = Wq.T[128k+p, 128h+e], with 1/E**0.25 folded in
    wq_s = np.asarray(Wq, dtype=np.float32) * np.float32(E ** -0.25)  # [oc, hin]
    wqT = np.ascontiguousarray(
        wq_s.reshape(H, E, KCH, 128).transpose(0, 3, 2, 1)  # [h, p, k, e]
    ).reshape(H, 128, KCH * E)

    keys_pad = np.zeros((NPAD, E), dtype=np.float32)
    keys_pad[:N] = np.asarray(keys, dtype=np.float32)
    keysT = np.ascontiguousarray(keys_pad.T).astype(bf)  # [E, NPAD]

    v_pad = np.zeros(NPAD, dtype=np.float32)
    v_pad[:N] = np.asarray(values, dtype=np.float32)
    mask = np.zeros(NPAD, dtype=np.float32)
    mask[:N] = 1.0

    v_hi32 = v_pad.astype(bf).astype(np.float32)
    v_lo32 = v_pad - v_hi32

    # vb[core][p, blk, VBW] with p = key index within 128-block;
    # cols: 0=v_hi, 1=v_lo, 2=mask, rest zero
    def shard_cols(a):  # [NPAD] -> [NCORES, 128, NBLK]
        return a.reshape(NCORES, NBLK, 128).transpose(0, 2, 1)

    vb = np.zeros((NCORES, 128, NBLK, VBW), dtype=np.float32)
    vb[..., 0] = shard_cols(v_hi32)
    vb[..., 1] = shard_cols(v_lo32)
    vb[..., 2] = shard_cols(mask)
    vb = vb.astype(bf)

    in_maps = []
    for c in range(NCORES):
        in_maps.append(
            {
                "xt": xT,
                "wqt": wqT,
                "keyst": np.ascontiguousarray(keysT[:, c * KC:(c + 1) * KC]),
                "vb": np.ascontiguousarray(vb[c].reshape(128, NBLK * VBW)),
            }
        )
    return in_maps


def kernel(x, curiosity_score, Wq, keys, values):
    global LAST_RESULTS
    if TRACE:
        _install_ntff_hook()
    from concourse.bass_utils import run_bass_kernel_spmd

    key = (XACT, LAG, NDB)
    if key not in _cache:
        _cache[key] = _build()
    nc = _cache[key]

    in_maps = _prep_inputs(x, Wq, keys, values)

    res = run_bass_kernel_spmd(
        nc, in_maps, core_ids=list(range(NCORES)), trace=TRACE
    )
    LAST_RESULTS = res

    # nd_out[p, h, bt]: strip s rows 32s+{0,1,2} = num_hi, num_lo, den partials
    nd = np.stack(
        [np.asarray(res.results[c]["nd_out"], dtype=np.float64) for c in range(NCORES)]
    ).reshape(NCORES, 128, H, BT)
    num = np.zeros((H, BT))
    den = np.zeros((H, BT))
    for s in range(NSTRIP):
        num += nd[:, 32 * s + 0].sum(axis=0) + nd[:, 32 * s + 1].sum(axis=0)
        den += nd[:, 32 * s + 2].sum(axis=0)
    out = (num / den).mean(axis=0) + np.asarray(
        curiosity_score, dtype=np.float64
    ).reshape(BT)
    return out.astype(np.float32).reshape(B, T, 1)
